# revision 14
# baseline (speedup 1.0000x reference)
"""Trainium2 Bass kernel for nn_ConvolutionalGenerator (conv stack + 2 pixel-wise
self-attention layers). 8-core SPMD: attention queries sharded 512-per-core;
convs replicated; attention outputs exchanged via AllGather.

Self-contained: hardcodes all shapes. kernel(**inputs) -> (out, p1, p2).
"""
import numpy as np

import concourse.bass as bass
import concourse.mybir as mybir
from concourse import bacc
import concourse.tile as tile
from concourse.bass_utils import run_bass_kernel_spmd

N_CORES = 8
F2 = 4096           # 64*64 pixels
IB = F2 // N_CORES  # 512 queries per core
HP = 66             # padded image side
NPAD = HP * HP      # 4356
F16 = mybir.dt.float16
F32 = mybir.dt.float32
EPS = 1e-5


def _pack_host_inputs(inputs):
    f16 = np.float16
    d = {}
    for ln in (1, 2):
        wf, bf = inputs[f'wf{ln}'], inputs[f'bf{ln}']
        wg, bg = inputs[f'wg{ln}'], inputs[f'bg{ln}']
        wh, bh = inputs[f'wh{ln}'], inputs[f'bh{ln}']
        gamma = inputs[f'gamma{ln}']
        wf_r = wf.reshape(2, F2, 16).astype(np.float64)
        wg_r = wg.reshape(2, F2, 16).astype(np.float64)
        bf_r = bf.reshape(2, F2).astype(np.float64)
        bg_r = bg.reshape(2, F2).astype(np.float64)
        A_q = np.einsum('cia,cib->abi', wf_r, wg_r).reshape(256, F2)
        u = np.einsum('ci,cib->bi', bf_r, wg_r)   # pairs with XN[b]
        v = np.einsum('ci,cia->ai', bg_r, wf_r)   # pairs with XT[a]
        A_lin = np.concatenate([u, v], axis=0)
        d[f'aq{ln}'] = [np.ascontiguousarray(A_q[:, c * IB:(c + 1) * IB]).astype(f16)
                        for c in range(N_CORES)]
        d[f'alin{ln}'] = [np.ascontiguousarray(A_lin[:, c * IB:(c + 1) * IB]).astype(f16)
                          for c in range(N_CORES)]
        g = float(gamma[0])
        d[f'whT{ln}'] = np.ascontiguousarray((g * wh).T).astype(f16)
        d[f'gbh{ln}'] = np.tile((g * bh).astype(np.float32)[None, :], (128, 1))

    selT = np.zeros((2, 16, 128), np.float32)
    selN = np.zeros((2, 16, 128), np.float32)
    for r in range(256):
        c, m = divmod(r, 128)
        a, b = divmod(r, 16)
        selT[c, a, m] = 1.0
        selN[c, b, m] = 1.0
    d['selT'] = selT.astype(f16)
    d['selN'] = selN.astype(f16)

    def pack_conv(w, b):
        taps = []
        for dy in range(3):
            for dx in range(3):
                t = w[:, :, dy, dx].T.astype(np.float32)
                if (dy, dx) == (1, 1):
                    t = np.concatenate([t, b[None, :].astype(np.float32)], axis=0)
                taps.append(np.ascontiguousarray(t).astype(f16))
        return taps

    d['w1a_t'] = pack_conv(inputs['w1a'], inputs['b1a'])
    d['w1b_t'] = pack_conv(inputs['w1b'], inputs['b1b'])
    d['w2_t'] = pack_conv(inputs['w2'], inputs['b2'])
    d['wl_t'] = pack_conv(inputs['wl'], inputs['bl'])
    d['ident16'] = np.eye(128, dtype=f16)
    d['ones_pad'] = np.ones((1, 4356), f16)
    d['ident32'] = np.eye(128, dtype=np.float32)
    return d


def _build_program():
    nc = bacc.Bacc("TRN2", target_bir_lowering=False, debug=False,
                   num_devices=N_CORES)
    di = {}

    def din(name, shape, dt=F32):
        di[name] = nc.dram_tensor(name, list(shape), dt, kind="ExternalInput").ap()

    din('z', [3, F2])
    for ln in (1, 2):
        din(f'aq{ln}', [256, IB], F16)
        din(f'alin{ln}', [32, IB], F16)
        din(f'whT{ln}', [16, 16], F16)
        din(f'gbh{ln}', [128, 16])
    din('selT', [2, 16, 128], F16)
    din('selN', [2, 16, 128], F16)
    din('ident16', [128, 128], F16)
    din('ones_pad', [1, 4356], F16)
    din('ident32', [128, 128], F32)
    for cv, ktap, cout in (('w1a_t', 4, 16), ('w1b_t', 17, 16),
                           ('w2_t', 17, 16), ('wl_t', 17, 1)):
        for t in range(9):
            k = ktap if t == 4 else ktap - 1
            din(f'{cv}{t}', [k, cout], F16)
    for vn in ('s1a', 'o1a', 's1b', 'o1b', 's2', 'o2'):
        din(vn, [16, 1])

    p_out = {ln: nc.dram_tensor(f'p{ln}', [IB, F2], F32, kind="ExternalOutput").ap()
             for ln in (1, 2)}
    img_out = nc.dram_tensor('img', [4, 1024], F32, kind="ExternalOutput").ap()

    with tile.TileContext(nc) as tc:
        _kernel_body(tc, di, p_out, img_out)
    nc.compile()
    return nc


def _kernel_body(tc, di, p_out, img_out):
    nc = tc.nc
    import contextlib
    ctx = contextlib.ExitStack()
    sb = ctx.enter_context(tc.tile_pool(name="sb", bufs=1))
    sb2 = ctx.enter_context(tc.tile_pool(name="sb2", bufs=2))
    ps = ctx.enter_context(tc.tile_pool(name="ps", bufs=2, space="PSUM"))
    dram = ctx.enter_context(tc.tile_pool(name="dram", bufs=1, space="DRAM"))

    # ---------------- constants / small inputs ----------------
    ident = sb.tile([128, 128], F16, tag="ident")
    nc.sync.dma_start(out=ident, in_=di['ident16'])
    ident32 = sb.tile([128, 128], F32, tag="ident32")
    nc.sync.dma_start(out=ident32, in_=di['ident32'])
    eps_t = sb.tile([16, 1], F32, tag="eps")
    nc.vector.memset(eps_t, EPS)

    selT = [sb.tile([16, 128], F16, tag=f"selT{c}", name=f"selT{c}") for c in range(2)]
    selN = [sb.tile([16, 128], F16, tag=f"selN{c}", name=f"selN{c}") for c in range(2)]
    for c in range(2):
        nc.sync.dma_start(out=selT[c], in_=di['selT'][c])
        nc.sync.dma_start(out=selN[c], in_=di['selN'][c])

    small = {}
    for vn in ('s1a', 'o1a', 's1b', 'o1b', 's2', 'o2'):
        small[vn] = sb.tile([16, 1], F32, tag=vn, name=vn)
        nc.sync.dma_start(out=small[vn], in_=di[vn])
    for ln in (1, 2):
        small[f'whT{ln}'] = sb.tile([16, 16], F16, tag=f'whT{ln}', name=f'whT{ln}')
        nc.sync.dma_start(out=small[f'whT{ln}'], in_=di[f'whT{ln}'])
        small[f'gbh{ln}'] = sb.tile([128, 16], F32, tag=f'gbh{ln}', name=f'gbh{ln}')
        nc.sync.dma_start(out=small[f'gbh{ln}'], in_=di[f'gbh{ln}'])

    conv_w = {}
    for cv, ktap, cout in (('w1a_t', 4, 16), ('w1b_t', 17, 16),
                           ('w2_t', 17, 16), ('wl_t', 17, 1)):
        for t in range(9):
            k = ktap if t == 4 else ktap - 1
            conv_w[(cv, t)] = sb.tile([k, cout], F16, tag=f'{cv}{t}', name=f'{cv}{t}')
            nc.sync.dma_start(out=conv_w[(cv, t)], in_=di[f'{cv}{t}'])

    aq, alin = {}, {}
    for ln in (1, 2):
        for c in range(2):
            aq[(ln, c)] = sb.tile([128, IB], F16, tag=f"aq{ln}_{c}", name=f"aq{ln}_{c}")
            nc.sync.dma_start(out=aq[(ln, c)], in_=di[f'aq{ln}'][128 * c:128 * (c + 1), :])
        for half, nm in ((0, 'N'), (1, 'T')):
            alin[(ln, nm)] = sb.tile([16, IB], F16, tag=f"alin{ln}{nm}",
                                     name=f"alin{ln}{nm}")
            nc.sync.dma_start(out=alin[(ln, nm)],
                              in_=di[f'alin{ln}'][16 * half:16 * (half + 1), :])

    # padded conv-input tiles; pad slots shared: A: conv1a/conv2, B: conv1b/convL
    def make_pad(tag, cin):
        t = sb.tile([17, NPAD], F16, tag=tag, name=tag)
        nc.vector.memset(t, 0.0)
        nc.sync.dma_start(out=t[cin:cin + 1, :], in_=di['ones_pad'])
        return t

    XX = {}
    XT = {}

    # conv1a input: z fp32 -> fp16 into pad interior
    pad_a = make_pad("padA", 3)
    zsb = sb.tile([16, F2], F32, tag="cwork")   # shared slot with conv work
    nc.sync.dma_start(out=zsb[0:3, :], in_=di['z'])
    nc.vector.tensor_copy(
        out=pad_a.rearrange("p (h w) -> p h w", h=HP)[0:3, 1:65, 1:65],
        in_=zsb[0:3, :].rearrange("p (h w) -> p h w", h=64))

    # ---------------- conv + bn + lrelu ----------------
    def conv_bn_lrelu(cv, pad_in, s_t, o_t, out_writer, convname):
        pv = pad_in.rearrange("p (h w) -> p h w", h=HP)
        cpsum = []
        for p in range(2):
            bank = ps.tile([128, 512], F32, tag="acc", name=f"cps{p}")
            cpsum.append(bank)
            for g in range(4):
                chk = 4 * p + g
                for t in range(9):
                    dy, dx = divmod(t, 3)
                    k = conv_w[(cv, t)].shape[0]
                    rhs = pv[0:k, dy + 8 * chk: dy + 8 * chk + 8, dx:dx + 64]
                    nc.tensor.matmul(
                        bank[32 * g:32 * g + 16, :], conv_w[(cv, t)], rhs,
                        start=(t == 0), stop=(t == 8),
                        tile_position=(0, 32 * g))
        cout_sb = sb.tile([16, F2], F32, tag="cwork")
        for p in range(2):
            for g in range(4):
                chk = 4 * p + g
                nc.any.tensor_copy(out=cout_sb[:, 512 * chk:512 * (chk + 1)],
                                   in_=cpsum[p][32 * g:32 * g + 16, :])
        stats = sb.tile([16, 8, nc.vector.BN_STATS_DIM], F32, tag="cstats")
        cr = cout_sb.rearrange("p (n f) -> p n f", f=512)
        for i in range(8):
            nc.vector.bn_stats(out=stats[:, i, :], in_=cr[:, i, :])
        mv = sb.tile([16, nc.vector.BN_AGGR_DIM], F32, tag="cmv")
        nc.vector.bn_aggr(out=mv, in_=stats)
        # a = s/sqrt(var+eps); b = o - a*mean
        std = sb.tile([16, 1], F32, tag="cstd")
        nc.scalar.activation(out=std, in_=mv[:, 1:2],
                             func=mybir.ActivationFunctionType.Sqrt,
                             bias=eps_t, scale=1.0)
        a_c = sb.tile([16, 1], F32, tag="cac")
        nc.vector.reciprocal(out=a_c, in_=std)
        nc.vector.tensor_mul(a_c, a_c, s_t)
        am = sb.tile([16, 1], F32, tag="cam")
        nc.vector.tensor_mul(am, a_c, mv[:, 0:1])
        b_c = sb.tile([16, 1], F32, tag="cbc")
        nc.vector.tensor_sub(b_c, o_t, am)
        y_aff = sb.tile([16, F2], F16, tag="cyaff")
        nc.scalar.activation(out=y_aff, in_=cout_sb,
                             func=mybir.ActivationFunctionType.Identity,
                             bias=b_c, scale=a_c)
        out_writer(y_aff)

    def lrelu_to(out_ap, y_aff):
        nc.vector.scalar_tensor_tensor(out=out_ap, in0=y_aff, scalar=0.1,
                                       in1=y_aff, op0=mybir.AluOpType.mult,
                                       op1=mybir.AluOpType.max)

    def writer_to_pad(pad_t):
        def w(y_aff):
            pv = pad_t.rearrange("p (h w) -> p h w", h=HP)
            lrelu_to(pv[0:16, 1:65, 1:65], y_aff)
        return w

    def writer_to_xx(ln):
        def w(y_aff):
            lrelu_to(XX[ln], y_aff)
            nc.vector.tensor_copy(
                out=XT[ln],
                in_=XX[ln].rearrange("p (h w) -> p w h", h=64))
        return w

    # ---------------- attention layer ----------------
    def attn_layer(ln, pout, res_pad_out):
        xx = XX[ln]
        xt = XT[ln]
        km = [sb.tile([128, F2], F16, tag=f"km{c}", name=f"km{c}") for c in range(2)]
        for c in range(2):
            for jc in range(8):
                js = slice(512 * jc, 512 * (jc + 1))
                pT = ps.tile([128, 512], F32, tag="repT", bufs=1)
                pN = ps.tile([128, 512], F32, tag="repN", bufs=1)
                nc.tensor.matmul(pT, selT[c], xt[:, js], start=True, stop=True)
                nc.tensor.matmul(pN, selN[c], xx[:, js], start=True, stop=True)
                xnrep = sb2.tile([128, 512], F16, tag="xnrep")
                nc.any.tensor_copy(out=xnrep, in_=pN)
                nc.vector.tensor_mul(km[c][:, js], pT, xnrep)
        # h (gamma folded) and its transpose hT [128, 32, 16]
        h16 = sb.tile([16, F2], F16, tag="h16")
        for jc in range(8):
            js = slice(512 * jc, 512 * (jc + 1))
            ph = ps.tile([16, 512], F32, tag="sc")
            nc.tensor.matmul(ph, small[f'whT{ln}'], xx[:, js],
                             start=True, stop=True)
            nc.any.tensor_copy(out=h16[:, js], in_=ph)
        hT = sb.tile([128, 32, 16], F16, tag="hT")
        for b4 in range(8):
            pt = ps.tile([128, 4, 16], F16, tag="ptr")
            for q in range(4):
                jc = 4 * b4 + q
                nc.tensor.transpose(pt[:, q, :], h16[:, 128 * jc:128 * (jc + 1)],
                                    ident[0:16, 0:16])
            nc.any.tensor_copy(out=hT[:, 4 * b4:4 * b4 + 4, :], in_=pt)

        ET = sb.tile([128, 32, 512], F16, tag="ET")
        zrec = []
        for it in range(4):
            E_t = sb2.tile([128, F2], F16, tag="E")
            zpart = sb.tile([128, 8], F32, tag=f"zpart{it}", name=f"zpart{it}")
            ofs = 128 * it
            for jc in range(8):
                js = slice(512 * jc, 512 * (jc + 1))
                sc = ps.tile([128, 512], F32, tag="sc")
                nc.tensor.matmul(sc, aq[(ln, 0)][:, ofs:ofs + 128], km[0][:, js],
                                 start=True, stop=False)
                nc.tensor.matmul(sc, aq[(ln, 1)][:, ofs:ofs + 128], km[1][:, js],
                                 start=False, stop=False)
                nc.tensor.matmul(sc, alin[(ln, 'N')][:, ofs:ofs + 128], xx[:, js],
                                 start=False, stop=False)
                nc.tensor.matmul(sc, alin[(ln, 'T')][:, ofs:ofs + 128], xt[:, js],
                                 start=False, stop=True)
                nc.scalar.activation(out=E_t[:, js], in_=sc,
                                     func=mybir.ActivationFunctionType.Exp,
                                     accum_out=zpart[:, jc:jc + 1])
            z_i = sb.tile([128, 1], F32, tag=f"z{it}", name=f"z{it}")
            nc.vector.tensor_reduce(out=z_i, in_=zpart, axis=mybir.AxisListType.X,
                                    op=mybir.AluOpType.add)
            rz = sb.tile([128, 1], F32, tag=f"rz{it}", name=f"rz{it}")
            nc.vector.reciprocal(out=rz, in_=z_i)
            zrec.append(rz)
            # attention-map rows out
            for hh in range(2):
                prow = sb2.tile([128, 2048], F32, tag="prow")
                nc.vector.tensor_scalar_mul(prow, E_t[:, 2048 * hh:2048 * (hh + 1)], rz)
                nc.sync.dma_start(out=pout[128 * it:128 * (it + 1),
                                           2048 * hh:2048 * (hh + 1)], in_=prow)
            # ET transposes for this i-tile
            for b8 in range(4):
                pt = ps.tile([128, 8, 128], F16, tag="ptr")
                for q in range(8):
                    jc = 8 * b8 + q
                    nc.tensor.transpose(pt[:, q, :],
                                        E_t[:, 128 * jc:128 * (jc + 1)], ident)
                nc.any.tensor_copy(
                    out=ET[:, 8 * b8:8 * b8 + 8, 128 * it:128 * (it + 1)], in_=pt)

        pU = ps.tile([16, 512], F32, tag="acc", name="pU")
        for jc in range(32):
            nc.tensor.matmul(pU, hT[:, jc, :], ET[:, jc, :],
                             start=(jc == 0), stop=(jc == 31))
        Usb = sb.tile([16, 512], F32, tag="Usb")
        nc.any.tensor_copy(out=Usb, in_=pU)
        pUT = ps.tile([128, 4, 16], F32, tag="ptr", name="pUT")
        for it in range(4):
            nc.tensor.transpose(pUT[:, it, :], Usb[:, 128 * it:128 * (it + 1)],
                                ident32[0:16, 0:16])
        ag_src = dram.tile([IB, 16], F32)
        ag_dst = dram.tile([F2, 16], F32)
        for it in range(4):
            yb = sb.tile([128, 16], F32, tag=f"yb{it}", name=f"yb{it}")
            nc.vector.scalar_tensor_tensor(out=yb, in0=pUT[:, it, :],
                                           scalar=zrec[it],
                                           in1=small[f'gbh{ln}'],
                                           op0=mybir.AluOpType.mult,
                                           op1=mybir.AluOpType.add)
            nc.sync.dma_start(out=ag_src[128 * it:128 * (it + 1), :], in_=yb)
        nc.gpsimd.collective_compute(
            "AllGather", mybir.AluOpType.bypass,
            ins=[ag_src.opt()], outs=[ag_dst.opt()],
            replica_groups=[list(range(N_CORES))],
        )
        go = sb.tile([128, 32, 16], F32, tag="go")
        nc.sync.dma_start(out=go, in_=ag_dst.rearrange("(c p) h -> p c h", p=128))
        rpv = res_pad_out.rearrange("p (h w) -> p h w", h=HP)
        for b4 in range(8):
            pt = ps.tile([16, 4, 128], F32, tag="ptr", name="pyT")
            for q in range(4):
                jc = 4 * b4 + q
                nc.tensor.transpose(pt[:, q, :], go[:, jc, :], ident32)
            nc.vector.tensor_tensor(
                out=rpv[0:16, 1 + 8 * b4:9 + 8 * b4, 1:65],
                in0=pt.rearrange("p a b -> p (a b)"),
                in1=xx[:, 512 * b4:512 * (b4 + 1)],
                op=mybir.AluOpType.add)

    # ---------------- program ----------------
    pad_b = make_pad("padB", 16)
    conv_bn_lrelu('w1a_t', pad_a, small['s1a'], small['o1a'],
                  writer_to_pad(pad_b), "1a")
    XX[1] = sb.tile([16, F2], F16, tag="XX1", name="XX1")
    XT[1] = sb.tile([16, F2], F16, tag="XT1", name="XT1")
    conv_bn_lrelu('w1b_t', pad_b, small['s1b'], small['o1b'],
                  writer_to_xx(1), "1b")
    pad_2 = make_pad("padA", 16)
    attn_layer(1, p_out[1], pad_2)
    XX[2] = sb.tile([16, F2], F16, tag="XX2", name="XX2")
    XT[2] = sb.tile([16, F2], F16, tag="XT2", name="XT2")
    conv_bn_lrelu('w2_t', pad_2, small['s2'], small['o2'],
                  writer_to_xx(2), "2")
    pad_L = make_pad("padB", 16)
    attn_layer(2, p_out[2], pad_L)

    # final conv + tanh
    pLv = pad_L.rearrange("p (h w) -> p h w", h=HP)
    outsb = sb.tile([128, 1024], F32, tag="outsb")
    for p in range(2):
        bank = ps.tile([128, 512], F32, tag="acc", name=f"cpsL{p}")
        for g in range(4):
            chk = 4 * p + g
            for t in range(9):
                dy, dx = divmod(t, 3)
                k = conv_w[('wl_t', t)].shape[0]
                rhs = pLv[0:k, dy + 8 * chk: dy + 8 * chk + 8, dx:dx + 64]
                nc.tensor.matmul(bank[32 * g:32 * g + 1, :], conv_w[('wl_t', t)],
                                 rhs, start=(t == 0), stop=(t == 8),
                                 tile_position=(0, 32 * g))
        for g in range(4):
            chk = 4 * p + g
            nc.scalar.activation(out=outsb[32 * g:32 * g + 1,
                                           512 * p:512 * (p + 1)],
                                 in_=bank[32 * g:32 * g + 1, :],
                                 func=mybir.ActivationFunctionType.Tanh)
    # outsb[32g, 512p:512(p+1)] holds chunk (4p+g). img flat chunk index = 4p+g.
    # Write per chunk: img row r=chk//2, col-half chk%2.
    for p in range(2):
        for g in range(4):
            chk = 4 * p + g
            nc.sync.dma_start(
                out=img_out[chk // 2:chk // 2 + 1,
                            512 * (chk % 2):512 * (chk % 2 + 1)],
                in_=outsb[32 * g:32 * g + 1, 512 * p:512 * (p + 1)])
    ctx.close()


_PROG_CACHE = {}
LAST_RUN = {}


def kernel(**inputs):
    inputs = {k: np.asarray(v) for k, v in inputs.items()}
    host = _pack_host_inputs(inputs)
    if 'nc' not in _PROG_CACHE:
        _PROG_CACHE['nc'] = _build_program()
    nc = _PROG_CACHE['nc']

    base = {
        'z': np.ascontiguousarray(inputs['z'].reshape(3, F2)).astype(np.float32),
        'selT': host['selT'], 'selN': host['selN'],
        'ident16': host['ident16'], 'ident32': host['ident32'],
        'ones_pad': host['ones_pad'],
    }
    for ln in (1, 2):
        base[f'whT{ln}'] = host[f'whT{ln}']
        base[f'gbh{ln}'] = host[f'gbh{ln}'].astype(np.float32)
    for cv in ('w1a_t', 'w1b_t', 'w2_t', 'wl_t'):
        for t in range(9):
            base[f'{cv}{t}'] = host[cv][t]
    for vn in ('s1a', 'o1a', 's1b', 'o1b', 's2', 'o2'):
        base[vn] = inputs[vn].reshape(16, 1).astype(np.float32)

    in_maps = []
    for c in range(N_CORES):
        m = dict(base)
        for ln in (1, 2):
            m[f'aq{ln}'] = host[f'aq{ln}'][c]
            m[f'alin{ln}'] = host[f'alin{ln}'][c]
        in_maps.append(m)

    res = run_bass_kernel_spmd(nc, in_maps, core_ids=list(range(N_CORES)))
    LAST_RUN['res'] = res
    results = res.results
    p1 = np.concatenate([results[c]['p1'] for c in range(N_CORES)], axis=0)
    p2 = np.concatenate([results[c]['p2'] for c in range(N_CORES)], axis=0)
    out = results[0]['img'].reshape(1, 1, 64, 64)
    return (out.astype(np.float32),
            p1.reshape(1, 1, F2, F2).astype(np.float32),
            p2.reshape(1, 1, F2, F2).astype(np.float32))
